# revision 59
# baseline (speedup 1.0000x reference)
"""Multi-head causal self-attention (B=2, T=2048, C=1024, H=16) on 8 TRN2
NeuronCores.

Sharding: tensor-parallel over heads -- 2 heads per core, both batch elements
on every core.  qkv column-parallel (each core's 256 q/k columns + 128 v
columns of W_qkv, host pre-permuted so each head's Q/K land in the partition
halves the kernel wants), proj row-parallel (each core's 128 W_proj rows);
the 8 partial outputs are summed on the host, which also adds b_proj once.

Dataflow (all matmul operands bf16 -> 1 cycle/row on PE, f32 psum accum):

  x    host-converted to bf16; loaded pre-transposed by DMA-engine xbar
       transposes (dma_start_transpose), so there are NO PE transposes and
       no x staging in SBUF: xT chunk tiles [128, 8cb, 256t] arrive directly.
  GEMM1 qT/kT[f, t] = Wqk_slice^T @ x  (lhsT = W slice, rhs = xT chunk),
       evicted psum->SBUF with the (per-partition) qkv bias on DVE.
  V    computed NATURAL (token-major) straight from xT: per 128-token tile,
       v[t, f2] = xT_tile^T(as lhsT) @ Wv_slice(as rhs); eviction drops the
       two heads' halves into v_sb[., kb, h, .] (bf16).  Column 64 (h0) /
       column 0 (h1) of each v block holds constant 1.0: the AV matmul then
       emits the softmax denominator as an extra psum row for free.
  QK   scoresT[k, q] per head: lhsT = kT slice (64 partitions = head's
       feature half), rhs = qT slice; both heads' matmuls use disjoint
       partition halves (auto tile_position row-tiling).
  softmax: scores in [-8.3, 8.3] for these inputs, so exp() needs no
       max-subtraction: one ACT pass psum->bf16 SBUF, scale=1/8, covering
       both heads ([128, 2, 512] per k-block).  Diagonal-crossing blocks get
       a DVE mask multiply (bf16 2x mode); above-diagonal blocks are never
       computed.
  AV   per head: av[0:65] = [V|1]^T @ attT (psum row 64 = denominator;
       matmul output base partitions must be 0/32/64, so both heads use
       rows 0:65 of their own psum bank).
  norm per (qc): both avs evicted into one st tile (DVE, frees the psum
       banks), denominator row staged to partition 0 and broadcast by
       gpsimd (partition-shifting tensor_copy + one partition_broadcast
       covering both heads), one DVE reciprocal, then h0's multiply writes
       aoT[0:64] directly and h1's goes via a small gpsimd partition-shift
       copy into aoT[64:128].
  GEMM2 out[t, c] = aoT(as lhsT) @ Wproj_slice; psum evicted to bf16 osb
       tiles (DVE; ACT helps once its exp stream is done) and stored with
       per-128-token DMAs on the sync queue; b_proj is added on the host.

Scheduling: emission order is queue order (in-order engines).  Attention is
software-pipelined two k-blocks deep (AV(kb) emitted after QK(kb+2)) and the
independent GEMM1/V chunk chains + GEMM2 tiles are spread as PE "fillers"
between k-block steps so the PE never starves while ACT runs exp.  Batch 0's
attention starts after only two GEMM1 chunks (its first q-chunk needs just
512 tokens of K/V); phase-A work for b1 fills b0's attention; each block's
GEMM2 fills the NEXT attention block (front 30-85% of its steps, drained
before the norm so the norm chain meets clean queues).  x transposes are
emitted lazily (4 chunks ahead) so output stores interleave with them on
the sync queue; const loads ride the scalar queue; GEMM2 evictions ride
DVE (ACT joins only once its exp stream is finished); the final GEMM2
drain also borrows the idle qk psum banks.
"""

from collections import deque

import numpy as np
import ml_dtypes

import concourse.bass as bass
import concourse.tile as tile
from concourse import bacc, mybir
from concourse.bass_utils import run_bass_kernel_spmd

P = 128
B, T, C, H, HD = 2, 2048, 1024, 16, 64
NCORES = 8
HPC = H // NCORES        # heads per core = 2
QC = 512                 # q-chunk
KB = 128                 # k-block
TC = 256                 # token chunk for GEMM1/V phase
MM_MODE = "bf16"         # kept for test.py compatibility

f32 = mybir.dt.float32
bf = mybir.dt.bfloat16
AF = mybir.ActivationFunctionType
ALU = mybir.AluOpType


def _build(tc_, x, wqk, bqk, wv, wproj, masks, out, Tloc):
    nc = tc_.nc
    BT = B * Tloc
    NTB = Tloc // TC         # GEMM1 token chunks per batch = 8
    NQ = Tloc // QC          # q-chunks per batch = 4
    NK = Tloc // KB          # k-blocks per batch = 16
    KPQ = QC // KB           # k-blocks per q-chunk = 4

    import contextlib
    ctx = contextlib.ExitStack()
    with ctx:
        consts = ctx.enter_context(tc_.tile_pool(name="consts", bufs=1))
        persist = ctx.enter_context(tc_.tile_pool(name="persist", bufs=1))
        xtp = ctx.enter_context(tc_.tile_pool(name="xtp", bufs=4))
        attp = ctx.enter_context(tc_.tile_pool(name="attp", bufs=6))
        bcp = ctx.enter_context(tc_.tile_pool(name="bcp", bufs=2))
        rsp = ctx.enter_context(tc_.tile_pool(name="rsp", bufs=4))
        tmp = ctx.enter_context(tc_.tile_pool(name="tmp", bufs=2))
        stp = ctx.enter_context(tc_.tile_pool(name="stp", bufs=2))
        outp = ctx.enter_context(tc_.tile_pool(name="outp", bufs=2))
        ps = ctx.enter_context(tc_.tile_pool(name="ps", bufs=2, space="PSUM"))
        psqk = ctx.enter_context(tc_.tile_pool(name="psqk", bufs=2, space="PSUM"))
        psav = ctx.enter_context(tc_.tile_pool(name="psav", bufs=1, space="PSUM"))

        # ---- constants: interleaved with the x transposes on the sync
        # queue, in first-use order, so the single DMA device serves the
        # first GEMM1 chunk's dependencies first ----
        wqk_sb = consts.tile([P, C // P, 2, P], bf)
        bqk_sb = consts.tile([P, 2], f32)
        wv_sb = consts.tile([P, C // P, P], bf)
        w2_sb = consts.tile([P, C], bf)
        masks_sb = consts.tile([P, KPQ, QC], bf)

        qkvT = persist.tile([P, 2, BT], bf)      # q/k feature-major
        aoT = persist.tile([P, BT], bf)          # attn out (normalized)
        v_sb = [persist.tile([P, NK, HPC, HD + 1], bf, name=f"v{b}")
                for b in range(B)]
        for b in range(B):
            # trailing ones column -> AV psum row 64 = softmax denominator
            nc.gpsimd.memset(v_sb[b][:, :, :, HD:HD + 1], 1.0)

        # ---- x loads: xbar-transposed straight into [c, t] tiles.
        # Emitted lazily (4 chunks ahead of consumption) so the later
        # output-store DMAs interleave with them on the sync queue instead
        # of queuing behind all 16 transposes. ----
        x_tiles = {}

        def ensure_xt(ti):
            if ti in x_tiles or ti >= B * NTB:
                return
            t0 = ti * TC
            xt = xtp.tile([P, C // P, TC], bf, name="xt")
            nc.sync.dma_start_transpose(xt, x[t0:t0 + TC, :])
            x_tiles[ti] = xt

        def ensure_xt_half(ti, half):
            """Split first-chunk load: halves arrive ~0.9us apart so the
            first GEMM1 chain starts earlier."""
            t0 = ti * TC
            hb = (C // P) // 2
            if ti not in x_tiles:
                x_tiles[ti] = xtp.tile([P, C // P, TC], bf, name="xt")
            xt = x_tiles[ti]
            cs = slice(half * hb * P, (half + 1) * hb * P)
            nc.sync.dma_start_transpose(
                xt[:, half * hb:(half + 1) * hb, :], x[t0:t0 + TC, cs])

        del ensure_xt_half
        nc.scalar.dma_start(out=wqk_sb, in_=wqk)
        nc.scalar.dma_start(out=bqk_sb, in_=bqk)
        for ti in range(4):
            ensure_xt(ti)
        xt_ahead = [4]

        # ---- phase A thunks: GEMM1 (q,k) + natural V for one chunk ----
        def a_gemm1(b, tib, bb):
            ti = b * NTB + tib
            t0 = ti * TC
            if bb == 0:
                ensure_xt(xt_ahead[0])
                xt_ahead[0] += 1
            xt = x_tiles[ti]
            g1 = ps.tile([P, TC], f32, tag="gemm", name="g1")
            for cb in range(C // P):
                nc.tensor.matmul(g1, wqk_sb[:, cb, bb, :], xt[:, cb, :],
                                 start=(cb == 0), stop=(cb == C // P - 1))
            nc.vector.tensor_scalar_add(
                out=qkvT[:, bb, t0:t0 + TC], in0=g1,
                scalar1=bqk_sb[:, bb:bb + 1])

        def a_v(b, tib):
            ti = b * NTB + tib
            xt = x_tiles[ti]
            vd = ps.tile([P, 2, P], f32, tag="gemm", name="vd")
            for a in range(2):
                for cb in range(C // P):
                    nc.tensor.matmul(
                        vd[:, a, :], xt[:, cb, a * P:(a + 1) * P],
                        wv_sb[:, cb, :],
                        start=(cb == 0), stop=(cb == C // P - 1))
            kb0 = tib * 2
            nc.vector.tensor_copy(
                out=v_sb[b][:, kb0:kb0 + 2, :, 0:HD], in_=vd)

        def chunk_thunks(b, tib):
            return [lambda: a_gemm1(b, tib, 0),
                    lambda: a_gemm1(b, tib, 1),
                    lambda: a_v(b, tib)]

        # ---- GEMM2 thunks for one (b, qc): 4 token tiles x 2 col halves,
        # evicted bf16 into one osb tile, stored with a single 2MB DMA ----
        NA = QC // P
        def g2_tile(b, qc, a, osb, act_ok, deep):
            tt0 = b * Tloc + qc * QC + a * P
            for ch in range(2):
                # post-loop GEMM2 also draws on the idle qk psum banks for
                # a deeper pipeline during the drain
                if deep and ch == 1:
                    g2 = psqk.tile([P, QC], f32, tag="qk", name="g2q")
                else:
                    g2 = ps.tile([P, QC], f32, tag="gemm", name="g2")
                nc.tensor.matmul(
                    g2, aoT[:, tt0:tt0 + P],
                    w2_sb[:, ch * QC:(ch + 1) * QC])
                # while exp() still streams, evictions stay OFF the scalar
                # queue (an eviction between exp calls delays every exp);
                # once attention is done ACT is free and doubles the rate
                dst = osb[:, a, ch * QC:(ch + 1) * QC]
                if act_ok and ch == 1:
                    nc.scalar.copy(out=dst, in_=g2)
                else:
                    nc.vector.tensor_copy(out=dst, in_=g2)
            nc.sync.dma_start(out=out[tt0:tt0 + P, :], in_=osb[:, a, :])

        def g2_thunks(b, qc, act_ok=False, deep=False):
            osb = outp.tile([P, NA, C], bf, name="osb")
            return [lambda a=a: g2_tile(b, qc, a, osb, act_ok, deep)
                    for a in range(NA)]

        # ---- attention for one (b, qc), fillers interleaved: `early`
        # (next block's phase-A chunks) front-loaded into the first 60% of
        # steps, `late` (GEMM2 tiles) spread across the whole block ----
        def attn_qc(b, qc, early, late):
            bt0 = b * Tloc
            q0 = bt0 + qc * QC
            nkb = KPQ * (qc + 1)
            av0 = psav.tile([P, QC], f32, tag="av0", name="av0")
            av1 = psav.tile([P, QC], f32, tag="av1", name="av1")
            atts = {}
            ne, nl = 0, 0
            nsteps = nkb + 2
            esteps = max(1, (nsteps * 2) // 5)

            lstart = nsteps * 3 // 10
            lend = max(lstart + 1, nkb - 2)   # drained before the norm chain

            def pump(step):
                nonlocal ne, nl
                want = min(len(early), ((step + 1) * len(early) + esteps - 1)
                           // esteps)
                while ne < want:
                    early[ne]()
                    ne += 1
                # late fillers start ~30% in (their norm inputs are fresh)
                # and finish early so the norm chain gets clean queues
                lstep = step - lstart
                want = max(0, min(len(late),
                                  ((lstep + 1) * len(late)) // (lend - lstart)))
                while nl < want:
                    late[nl]()
                    nl += 1

            for step in range(nsteps):
                if step < nkb:
                    kb = step
                    ks = slice(bt0 + kb * KB, bt0 + (kb + 1) * KB)
                    q_lo = max(0, (kb - KPQ * qc) * KB)
                    qk = psqk.tile([P, 2, QC], f32, tag="qk", name="qk")
                    for h in range(HPC):
                        hs = slice(HD * h, HD * (h + 1))
                        nc.tensor.matmul(
                            qk[:, h, q_lo:QC], qkvT[hs, 1, ks],
                            qkvT[hs, 0, q0 + q_lo:q0 + QC])
                    att = attp.tile([P, 2, QC], bf, tag="att", name="att")
                    nc.scalar.activation(
                        out=att[:, :, q_lo:QC], in_=qk[:, :, q_lo:QC],
                        func=AF.Exp, scale=1.0 / 8.0)
                    if kb >= KPQ * qc:          # diagonal-crossing block:
                        # only q in [q_lo, q_lo+KB) is partially masked
                        joff = kb - KPQ * qc
                        mw = KB * (joff + 1)
                        nc.vector.tensor_mul(
                            out=att[:, :, q_lo:mw], in0=att[:, :, q_lo:mw],
                            in1=masks_sb[:, joff:joff + 1, q_lo:mw]
                            .to_broadcast((P, 2, mw - q_lo)))
                    atts[kb] = att
                if step >= 2:
                    kb = step - 2
                    att = atts.pop(kb)
                    q_lo = max(0, (kb - KPQ * qc) * KB)
                    for h, av in ((0, av0), (1, av1)):
                        nc.tensor.matmul(
                            av[0:HD + 1, q_lo:QC], v_sb[b][:, kb, h, :],
                            att[:, h, q_lo:QC],
                            start=(kb == 0), stop=(kb == nkb - 1))
                if step < nkb:
                    pump(step)

            # normalization -> aoT (h0 direct; h1 partition-shifted by DMA)
            st = stp.tile([HD + 1, 2, QC], f32, tag="st", name="st")
            nc.vector.tensor_copy(out=st[:, 0, :], in_=av0[0:HD + 1, :])
            nc.vector.tensor_copy(out=st[:, 1, :], in_=av1[0:HD + 1, :])
            rs = rsp.tile([1, 2, QC], f32, tag="rs", name="rs")
            nc.gpsimd.tensor_copy(out=rs, in_=st[HD:HD + 1, :, :])
            bc = bcp.tile([HD, 2, QC], f32, tag="bc", name="bc")
            nc.gpsimd.partition_broadcast(bc, rs, channels=HD)
            bcr = bcp.tile([HD, 2, QC], f32, tag="bcr", name="bcr")
            nc.vector.reciprocal_approx_fast(out=bcr, in_=bc)
            nc.vector.tensor_mul(
                out=aoT[0:HD, q0:q0 + QC], in0=st[0:HD, 0, :],
                in1=bcr[:, 0, :])
            # h1: one gpsimd multiply that also shifts partitions 0:64 ->
            # 64:128 (SBUF-only operands; BIR rejects gpsimd PSUM reads)
            nc.gpsimd.tensor_mul(
                out=aoT[HD:P, q0:q0 + QC], in0=st[0:HD, 1, :],
                in1=bcr[:, 1, :])
            # leftover fillers drain AFTER the norm chain is queued, so
            # their engine work never delays the next block / final GEMM2
            while ne < len(early):
                early[ne]()
                ne += 1
            while nl < len(late):
                late[nl]()
                nl += 1

        # ---- schedule: attention blocks with look-ahead fillers ----
        CPQ = QC // TC           # phase-A chunks feeding one q-chunk = 2
        seq = [(0, qc) for qc in range(NQ)] + \
              [(1, qc) for qc in ([1, 2, 3, 0] if NQ == 4 else range(NQ))]
        emitted = {b: 0 for b in range(B)}   # phase-A chunks emitted so far

        def chunks_upto(b, hi):
            ths = []
            while emitted[b] < min(hi, NTB):
                ths += chunk_thunks(b, emitted[b])
                emitted[b] += 1
            return ths

        nc.scalar.dma_start(out=wv_sb, in_=wv)
        for th in chunks_upto(0, CPQ):       # preamble: b0 qc0's K/V
            th()
        nc.scalar.dma_start(out=masks_sb, in_=masks)
        nc.scalar.dma_start(out=w2_sb, in_=wproj)
        for idx, (b, qc) in enumerate(seq):
            early, late = [], []
            if idx + 1 < len(seq):
                bn, qcn = seq[idx + 1]
                early += chunks_upto(bn, CPQ * (qcn + 1))
            if idx >= 1:
                late += g2_thunks(*seq[idx - 1],
                                  act_ok=(idx == len(seq) - 1))
            attn_qc(b, qc, early, late)
        for th in g2_thunks(*seq[-1], act_ok=True, deep=True):
            th()


def build_nc(Tloc=T, mm_mode=MM_MODE, dbg_taps=False, niter=1):
    del mm_mode, dbg_taps
    nc = bacc.Bacc("TRN2", target_bir_lowering=False, debug=False,
                   num_devices=NCORES)
    BT = B * Tloc
    KPQ = QC // KB
    x = nc.dram_tensor("x", [BT, C], bf, kind="ExternalInput").ap()
    wqk = nc.dram_tensor("wqk", [P, C // P, 2, P], bf,
                         kind="ExternalInput").ap()
    bqk = nc.dram_tensor("bqk", [P, 2], f32, kind="ExternalInput").ap()
    wv = nc.dram_tensor("wv", [P, C // P, P], bf, kind="ExternalInput").ap()
    wproj = nc.dram_tensor("wproj", [P, C], bf, kind="ExternalInput").ap()
    masks = nc.dram_tensor("masks", [P, KPQ, QC], bf,
                           kind="ExternalInput").ap()
    out = nc.dram_tensor("out", [BT, C], bf, kind="ExternalOutput").ap()
    with tile.TileContext(nc) as tc_:
        for _ in range(niter):
            _build(tc_, x, wqk, bqk, wv, wproj, masks, out, Tloc)
    nc.compile()
    return nc


def make_in_maps(x2d, W_qkv, b_qkv, W_proj, b_proj):
    """Per-core input dicts.  x is converted to bf16 once (shared across
    cores); W_qkv is column-sliced + pre-permuted into the SBUF layouts the
    kernel wants; W_proj is row-sliced.  b_proj is NOT shipped -- the host
    adds it after summing the 8 partial outputs."""
    del b_proj
    KPQ = QC // KB
    x_bf = np.ascontiguousarray(x2d.astype(ml_dtypes.bfloat16))
    pp = np.arange(P)
    jj = np.arange(P)
    mp, mj, mq = np.meshgrid(np.arange(P), np.arange(KPQ), np.arange(QC),
                             indexing="ij")
    masks = (mq >= mj * KB + mp).astype(ml_dtypes.bfloat16)
    in_maps = []
    for core in range(NCORES):
        qk_cols = np.empty((2, P), np.int64)
        for bb in range(2):
            qk_cols[bb] = (384 * core + 192 * (jj // HD) + HD * bb
                           + (jj % HD))
        v_cols = 384 * core + 192 * (jj // HD) + 2 * HD + (jj % HD)
        wqk = W_qkv[:, qk_cols.T].astype(ml_dtypes.bfloat16)   # [C, 128, 2]
        wqk = np.ascontiguousarray(
            wqk.reshape(C // P, P, P, 2).transpose(1, 0, 3, 2))
        bq = np.ascontiguousarray(
            b_qkv[qk_cols].T.astype(np.float32))               # [128, 2]
        wv = W_qkv[:, v_cols].astype(ml_dtypes.bfloat16)       # [C, 128]
        wv = np.ascontiguousarray(
            wv.reshape(C // P, P, P).transpose(1, 0, 2))
        wp = np.ascontiguousarray(
            W_proj[P * core:P * (core + 1), :].astype(ml_dtypes.bfloat16))
        in_maps.append({
            "x": x_bf, "wqk": wqk, "bqk": bq, "wv": wv, "wproj": wp,
            "masks": masks,
        })
    return in_maps


_NC_CACHE = {}


def _get_nc(Tloc=T, mm_mode=MM_MODE):
    key = (Tloc, mm_mode)
    if key not in _NC_CACHE:
        _NC_CACHE[key] = build_nc(Tloc, mm_mode)
    return _NC_CACHE[key]


def kernel(x, W_qkv, b_qkv, W_proj, b_proj):
    x2d = np.ascontiguousarray(
        np.asarray(x, np.float32).reshape(B * T, C))
    in_maps = make_in_maps(
        x2d, np.asarray(W_qkv), np.asarray(b_qkv),
        np.asarray(W_proj), np.asarray(b_proj))
    nc = _get_nc()
    res = run_bass_kernel_spmd(nc, in_maps, core_ids=list(range(NCORES)))
    acc = res.results[0]["out"].astype(np.float32)
    for i in range(1, NCORES):
        acc = acc + res.results[i]["out"]
    acc = acc + np.asarray(b_proj, np.float32)[None, :]
    return acc.reshape(B, T, C)
